# revision 12
# baseline (speedup 1.0000x reference)
"""LocalBandSimilarityBlock — Trainium2 Bass kernel, 8-way sequence-parallel.

Strategy: sort nodes by grid-x (host-side sharding permutation). After the
sort, every node's radius-2 neighbourhood lies within a +-H-row band in
sorted order (verified per-input on host; H is chosen per input as 192 or
256). Each of the 8 cores owns 768 sorted query rows plus an H-row halo on
each side, so cores run fully independently, no collectives. Per 128-query
block, attention is computed against a (2H+128)-row key window instead of
all 6144 keys.

v2 layout: x arrives HOST-TRANSPOSED (xT, feature-major) so the LN1
normalisation happens directly in the transposed layout used by every
matmul — no on-device DMA transposes for h/hn. Row statistics (mean /
mean-square) are computed with ones-matmuls on the tensor engine, which
yields them already broadcast across partitions; 1/sqrt is computed as
exp(-0.5*ln(var+eps)) so LN1, softmax and LN2 share a single activation
table set (ln+exp) and only gelu needs a second table load. Softmax runs
without a max-subtraction: the additive band mask (+PEN per matching grid
axis via one-hot matmuls, -PEN on the self diagonal) is recentred by an
exp bias of -2*PEN, making valid logits O(1) and masked logits <= -PEN.
The whole schedule is software-pipelined: projections begin as soon as the
first 512 transposed-LN columns are ready, attention blocks interleave
with the Wo/LN2 phase, and the FFN starts while the attention tail is
still draining.
"""
import os
import sys

import numpy as np

sys.path.insert(0, "/opt/trn_rl_repo")

import ml_dtypes  # noqa: E402
from contextlib import ExitStack  # noqa: E402

import concourse.bacc as bacc  # noqa: E402
import concourse.tile as tile  # noqa: E402
from concourse import mybir  # noqa: E402
from concourse.masks import make_identity  # noqa: E402
from concourse.tile_rust import add_dep_helper  # noqa: E402
from concourse.bass_utils import run_bass_kernel_spmd  # noqa: E402

P = 128
D = 512
DC = D // P            # 4 d-chunks
DFF = 2048
FC = DFF // P          # 16 ffn chunks
N = 6144
N_CORES = 8
ROWS = N // N_CORES    # 768 query rows per core
NB = ROWS // P         # 6 query blocks per core
GR = 384               # ffn row group (fits one PSUM bank)
PEN = 120.0
LN_EPS = 1e-5
RADIUS = 2
F32 = mybir.dt.float32
BF16 = mybir.dt.bfloat16
AX = mybir.AxisListType
OP = mybir.AluOpType
ACT = mybir.ActivationFunctionType
RNRB = float(-0.5 * np.log(D))   # exp bias turning rstd into 1/||h||

LAST_EXEC_NS = None


def build_program(H: int):
    KR = ROWS + 2 * H      # key rows per core
    NT = KR // P           # key chunks
    W = 2 * H + P          # key window per query block
    WSPANS = [(0, 512)] if W <= 512 else [(0, 512), (512, W - 512)]
    SL = [(s, min(512, KR - s)) for s in range(0, KR, 512)]  # stat col slices

    nc = bacc.Bacc("TRN2", target_bir_lowering=False, debug=False,
                   num_devices=N_CORES)
    dt_in = lambda name, shape, dt: nc.dram_tensor(name, shape, dt,
                                                   kind="ExternalInput").ap()
    xtb = dt_in("xtb", [P, DC, KR], BF16)    # host-transposed x (bf16)
    xq32 = dt_in("xq32", [ROWS, D], F32)     # own rows, row-major (residual)
    gxkb = dt_in("gxkb", [P, KR], BF16)
    gykb = dt_in("gykb", [P, KR], BF16)
    iota = dt_in("iota", [P, 1], F32)
    wq = dt_in("wq", [P, DC, D], BF16)       # pre-scaled by 1/sqrt(D) on host
    wk = dt_in("wk", [P, DC, D], BF16)
    wv = dt_in("wv", [P, DC, D], BF16)
    wo = dt_in("wo", [P, DC, D], BF16)
    w1 = dt_in("w1", [P, DC, DFF], BF16)
    w2 = dt_in("w2", [P, FC, D], BF16)
    bqs = dt_in("bqs", [P, DC], F32)         # (bq / sqrt(D)) chunked [p, dc]
    bks = dt_in("bks", [P, DC], F32)
    b1s = dt_in("b1s", [P, FC], F32)
    bvrow = dt_in("bvrow", [1, D], BF16)
    borow = dt_in("borow", [1, D], BF16)
    b2row = dt_in("b2row", [1, D], BF16)
    out = nc.dram_tensor("out", [ROWS, D], F32, kind="ExternalOutput").ap()

    with tile.TileContext(nc) as tc, ExitStack() as ctx:
        const = ctx.enter_context(tc.tile_pool(name="const", bufs=1))
        big = ctx.enter_context(tc.tile_pool(name="big", bufs=1))
        temps = ctx.enter_context(tc.tile_pool(name="temps", bufs=2))
        varp = ctx.enter_context(tc.tile_pool(name="varp", bufs=2))
        small = ctx.enter_context(tc.tile_pool(name="small", bufs=12))
        psA = ctx.enter_context(tc.tile_pool(name="psA", bufs=4, space="PSUM"))
        psS = ctx.enter_context(tc.tile_pool(name="psS", bufs=2, space="PSUM"))

        # float biases used by scalar.activation need registered const APs
        for val in (0.0, LN_EPS, RNRB, -2.0 * PEN):
            ct = const.tile([P, 1], F32, tag=f"const_{val}")
            nc.vector.memset(ct, val)
            nc.const_aps.aps[(F32, val)] = ct

        # ---- small constants + early DMAs
        bqs_t = const.tile([P, DC], F32)
        bks_t = const.tile([P, DC], F32)
        b1s_t = const.tile([P, FC], F32)
        bvrow_t = const.tile([1, D], BF16)
        borow_t = const.tile([1, D], BF16)
        b2row_t = const.tile([1, D], BF16)
        iota_t = const.tile([P, 1], F32)
        for t_, a_ in [(bqs_t, bqs), (bks_t, bks), (b1s_t, b1s),
                       (bvrow_t, bvrow), (borow_t, borow), (b2row_t, b2row),
                       (iota_t, iota)]:
            nc.sync.dma_start(out=t_, in_=a_)

        gx_st = temps.tile([P, KR], BF16, tag="gstage", bufs=2)
        gy_st = temps.tile([P, KR], BF16, tag="gstage", bufs=2)
        nc.sync.dma_start(out=gx_st, in_=gxkb)
        nc.sync.dma_start(out=gy_st, in_=gykb)

        # transposed input: 4 chunks, split columns to keep DMAs <= 256KB
        xtb_t = big.tile([P, DC, KR], BF16)
        hkr = KR // 2
        for c in range(DC):
            for h0, h1 in [(0, hkr), (hkr, KR)]:
                nc.sync.dma_start(out=xtb_t[:, c, h0:h1], in_=xtb[:, c, h0:h1])

        def load_split(eng, dst, srcap, max_bytes=262144):
            """DMA in <=256KB pieces (one HW sub-DMA each)."""
            nbytes = int(np.prod(dst.shape)) * mybir.dt.size(dst.dtype)
            if nbytes <= max_bytes or len(dst.shape) < 2:
                eng.dma_start(out=dst, in_=srcap)
                return
            nsplit = -(-nbytes // max_bytes)
            cnt = dst.shape[1]
            step = max(1, -(-cnt // nsplit))
            for s0 in range(0, cnt, step):
                s1 = min(cnt, s0 + step)
                if len(dst.shape) == 2:
                    eng.dma_start(out=dst[:, s0:s1], in_=srcap[:, s0:s1])
                else:
                    eng.dma_start(out=dst[:, s0:s1, :], in_=srcap[:, s0:s1, :])

        wq_t = const.tile([P, DC, D], BF16)
        wk_t = const.tile([P, DC, D], BF16)
        wv_t = const.tile([P, DC, D], BF16)
        wo_t = const.tile([P, DC, D], BF16)
        w1_t = const.tile([P, DC, DFF], BF16)
        w2_t = const.tile([P, FC, D], BF16)
        for t_, a_ in [(wk_t, wk), (wv_t, wv), (wq_t, wq)]:
            load_split(nc.scalar, t_, a_)

        # identity / shifted self-penalty / ones
        ident = const.tile([P, P], BF16)
        make_identity(nc, ident)
        shiftpen = const.tile([P, W], BF16)
        nc.gpsimd.memset(shiftpen, 0.0)
        nc.gpsimd.affine_select(out=shiftpen, in_=shiftpen,
                                compare_op=OP.not_equal, fill=-PEN,
                                base=H, channel_multiplier=1,
                                pattern=[[-1, W]])
        ones1 = const.tile([1, P], BF16)
        nc.vector.memset(ones1, 1.0)
        onesD = const.tile([P, P], BF16)
        nc.vector.memset(onesD, 1.0 / D)

        # ---- persistent activation tiles.  HT and the 4 band-mask tiles
        # pack into one [P, 8, KR] slab that gt later reuses (all are dead
        # once the last attention block's S accumulation has run).
        slabA = big.tile([P, 8, KR], BF16, tag="slotA")
        HT = slabA[:, 0:DC, :]
        ExPEN, EyPEN = slabA[:, 4, :], slabA[:, 5, :]
        Fx, Fy = slabA[:, 6, :], slabA[:, 7, :]
        gt = big.tile([P, FC, ROWS], BF16, tag="slotA")   # reuses slabA
        xsq = big.tile([P, DC, KR], BF16, tag="slotB")
        x2t = big.tile([P, NB, D], F32, tag="slotB")      # reuses xsq slot
        HnT = big.tile([P, DC, KR], BF16)

        # band-mask one-hot encodings (valid axis match -> +PEN via matmul)
        for src_bc, Et, Ft in [(gx_st, ExPEN, Fx), (gy_st, EyPEN, Fy)]:
            nc.vector.tensor_scalar(Et, src_bc, iota_t, PEN,
                                    op0=OP.is_equal, op1=OP.mult)
            u = temps.tile([P, KR], BF16, tag="uband")
            nc.gpsimd.tensor_scalar(u, src_bc, iota_t, None, op0=OP.subtract)
            a1 = temps.tile([P, KR], BF16, tag="uband")
            nc.gpsimd.tensor_scalar(a1, u, RADIUS + 0.5, None, op0=OP.is_le)
            nc.gpsimd.tensor_scalar(u, u, -(RADIUS + 0.5), None, op0=OP.is_ge)
            nc.gpsimd.tensor_tensor(Ft, a1, u, op=OP.mult)
        kt = big.tile([P, DC, KR], BF16)
        qt = big.tile([P, DC, ROWS], BF16)
        vt = big.tile([P, NT, D], BF16)
        aoT = big.tile([P, DC, ROWS], BF16)
        h2T = big.tile([P, DC, ROWS], BF16)
        mu_bc = big.tile([P, KR], BF16)
        rstd_bc = big.tile([P, KR], BF16)
        rnr_bc = big.tile([P, KR], BF16)
        var3a = small.tile([P, 3], F32, tag="var3a")
        var3b = small.tile([P, 3], F32, tag="var3b")
        mv6 = [small.tile([P, 2], F32, tag=f"mv6_{b}", name=f"mv6_{b}")
               for b in range(NB)]

        # x^2 for the mean-square statistic (gpsimd; vector is busier)
        for c in range(DC):
            nc.gpsimd.tensor_tensor(xsq[:, c, :], xtb_t[:, c, :],
                                    xtb_t[:, c, :], op=OP.mult)

        last_xsq_mm = [None]
        last_ht_mm = [None]
        last_hnt_mm = [None]

        # ---- LN1 in transposed land, per 512-column slice
        def ln_slice(s0, sn):
            mu_ps = psA.tile([P, 512], F32, tag="ps")
            for c in range(DC):
                nc.tensor.matmul(mu_ps[:, :sn], onesD,
                                 xtb_t[:, c, s0:s0 + sn],
                                 start=(c == 0), stop=(c == DC - 1))
            mq_ps = psA.tile([P, 512], F32, tag="ps")
            for c in range(DC):
                mm = nc.tensor.matmul(mq_ps[:, :sn], onesD,
                                      xsq[:, c, s0:s0 + sn],
                                      start=(c == 0), stop=(c == DC - 1))
                last_xsq_mm[0] = mm
            vs = varp.tile([P, 512], F32, tag="vs")
            nc.scalar.activation(vs[:, :sn], mu_ps[:, :sn], ACT.Square)
            nc.vector.tensor_tensor(vs[:, :sn], mq_ps[:, :sn], vs[:, :sn],
                                    op=OP.subtract)
            nc.vector.tensor_scalar_add(mu_bc[:, s0:s0 + sn], mu_ps[:, :sn],
                                        0.0)
            lnv = varp.tile([P, 512], F32, tag="lnv")
            nc.scalar.activation(lnv[:, :sn], vs[:, :sn], ACT.Ln, bias=LN_EPS)
            nc.scalar.activation(rstd_bc[:, s0:s0 + sn], lnv[:, :sn],
                                 ACT.Exp, scale=-0.5)
            nc.scalar.activation(rnr_bc[:, s0:s0 + sn], lnv[:, :sn],
                                 ACT.Exp, scale=-0.5, bias=RNRB)
            for c in range(DC):
                tcc = temps.tile([P, 512], BF16, tag="tcc")
                nc.vector.tensor_tensor(tcc[:, :sn], xtb_t[:, c, s0:s0 + sn],
                                        mu_bc[:, s0:s0 + sn], op=OP.subtract)
                nc.vector.tensor_tensor(HT[:, c, s0:s0 + sn], tcc[:, :sn],
                                        rstd_bc[:, s0:s0 + sn], op=OP.mult)
                nc.gpsimd.tensor_tensor(HnT[:, c, s0:s0 + sn], tcc[:, :sn],
                                        rnr_bc[:, s0:s0 + sn], op=OP.mult)

        def kq_rowblock(kind, r0, rn_):
            """q^T/k^T row-block: 4 dout-chunks x (4 accum + bias) matmuls."""
            src_t, dst, bias, off = ((wk_t, kt, bks_t, 0) if kind == "k"
                                     else (wq_t, qt, bqs_t, H))
            for dcx in range(DC):
                ps = psA.tile([P, 512], F32, tag="ps")
                for ci in range(DC):
                    mm = nc.tensor.matmul(ps[:, :rn_],
                                          src_t[:, ci, dcx * P:(dcx + 1) * P],
                                          HT[:, ci, off + r0:off + r0 + rn_],
                                          start=(ci == 0), stop=(ci == DC - 1))
                    last_ht_mm[0] = mm
                if kind == "k":
                    nc.vector.tensor_scalar(dst[:, dcx, r0:r0 + rn_],
                                            ps[:, :rn_], bias[:, dcx:dcx + 1],
                                            None, op0=OP.add)
                else:
                    nc.scalar.activation(dst[:, dcx, r0:r0 + rn_],
                                         ps[:, :rn_], ACT.Identity,
                                         bias=bias[:, dcx:dcx + 1], scale=1.0)

        def v_chunk(t):
            """v rows t*128..t*128+128 (row-major), bias folded as K=1 mm."""
            ps = psA.tile([P, 512], F32, tag="ps")
            for ci in range(DC):
                mm = nc.tensor.matmul(ps, HT[:, ci, t * P:(t + 1) * P],
                                      wv_t[:, ci, :], start=(ci == 0),
                                      stop=False)
                last_ht_mm[0] = mm
            nc.tensor.matmul(ps, ones1, bvrow_t, start=False, stop=True)
            nc.vector.tensor_scalar_add(vt[:, t, :], ps, 0.0)

        # pipelined emission: stats slice -> projections that only need it
        ln_slice(*SL[0])
        kq_rowblock("k", 0, 512)
        for t in range(0, 4):
            v_chunk(t)
        ln_slice(*SL[1])
        kq_rowblock("q", 0, 512)
        kq_rowblock("k", 512, 512)
        kq_rowblock("q", 512, 256)
        for t in range(4, 8):
            v_chunk(t)
        if len(SL) > 2:
            ln_slice(*SL[2])
        if KR > 1024:
            kq_rowblock("k", 1024, KR - 1024)
        for t in range(8, NT):
            v_chunk(t)
        load_split(nc.scalar, wo_t, wo)

        # ---- attention block: S accum -> exp(bias=-2PEN) -> P^T -> PV
        def attn_s(b):
            wb = P * b
            S = psS.tile([P, W], F32, tag="S")
            for c0, cn in WSPANS:
                nc.tensor.matmul(S[:, c0:c0 + cn], ident,
                                 shiftpen[:, c0:c0 + cn], start=True,
                                 stop=False)
                for Et, Ft in [(ExPEN, Fx), (EyPEN, Fy)]:
                    nc.tensor.matmul(S[:, c0:c0 + cn],
                                     Et[:, H + wb:H + wb + P],
                                     Ft[:, wb + c0:wb + c0 + cn],
                                     start=False, stop=False)
                for ac in range(2 * DC):
                    if ac < DC:
                        lhsT = qt[:, ac, wb:wb + P]
                        rhs = kt[:, ac, wb + c0:wb + c0 + cn]
                    else:
                        lhsT = HnT[:, ac - DC, H + wb:H + wb + P]
                        rhs = HnT[:, ac - DC, wb + c0:wb + c0 + cn]
                    mm = nc.tensor.matmul(
                        S[:, c0:c0 + cn], lhsT, rhs, start=False,
                        stop=(ac == 2 * DC - 1))
                    if ac >= DC:
                        last_hnt_mm[0] = mm
            pb = temps.tile([P, W], BF16, tag="pb")
            srow = small.tile([P, 1], F32, tag="srow")
            nc.scalar.activation(pb, S, ACT.Exp, bias=-2.0 * PEN, scale=1.0,
                                 accum_out=srow)
            rs = small.tile([P, 1], F32, tag="rs")
            nc.vector.reciprocal(rs, srow)
            PT = temps.tile([P, W // P, P], BF16, tag="PT")
            nc.sync.dma_start_transpose(PT[:, :, :], pb)
            return PT, rs

        def attn_pv(b, PT, rs):
            po = psA.tile([P, 512], F32, tag="ps")
            for j in range(W // P):
                nc.tensor.matmul(po, PT[:, j, :], vt[:, b + j, :],
                                 start=(j == 0), stop=(j == W // P - 1))
            aob = temps.tile([P, D], BF16, tag="aob")
            nc.vector.tensor_scalar_mul(aob, po, rs)
            nc.scalar.dma_start_transpose(aoT[:, :, b * P:(b + 1) * P], aob)

        def phase_d(b):
            """x2 = x + attn@Wo + bo, then LN2 stats (no scalar table ops)."""
            xr = temps.tile([P, D], F32, tag="xr")
            nc.gpsimd.dma_start(out=xr, in_=xq32[b * P:(b + 1) * P, :])
            ps = psA.tile([P, 512], F32, tag="ps")
            nc.tensor.matmul(ps, ones1, borow_t, start=True, stop=False)
            for ci in range(DC):
                nc.tensor.matmul(ps, aoT[:, ci, b * P:(b + 1) * P],
                                 wo_t[:, ci, :],
                                 start=False, stop=(ci == DC - 1))
            x2 = x2t[:, b, :]
            x2i = nc.vector.scalar_tensor_tensor(x2, ps, 0.0, xr,
                                                 op0=OP.bypass, op1=OP.add)
            add_dep_helper(x2i.ins, last_xsq_mm[0].ins, sync=True,
                           reason="x2t reuses xsq slot")
            st = small.tile([P, 6], F32, tag="st")
            nc.vector.bn_stats(st, x2)
            nc.vector.bn_aggr(mv6[b], st)
            dst = var3a if b < 3 else var3b
            nc.vector.tensor_scalar_add(dst[:, b % 3:b % 3 + 1],
                                        mv6[b][:, 1:2], 0.0)

        def ln2_batch(var3, blocks):
            lnv3 = small.tile([P, 3], F32, tag="lnv3")
            nc.scalar.activation(lnv3, var3, ACT.Ln, bias=LN_EPS)
            rstd3 = small.tile([P, 3], F32, tag="rstd3")
            nc.scalar.activation(rstd3, lnv3, ACT.Exp, scale=-0.5)
            for i, b in enumerate(blocks):
                h2b = temps.tile([P, D], BF16, tag="h2b")
                nc.vector.tensor_scalar(h2b, x2t[:, b, :], mv6[b][:, 0:1],
                                        rstd3[:, i:i + 1],
                                        op0=OP.subtract, op1=OP.mult)
                eng = nc.sync if b % 2 == 0 else nc.scalar
                eng.dma_start_transpose(h2T[:, :, b * P:(b + 1) * P], h2b)

        # ---- attention + phase D interleaved: PV lags one block behind S,
        # the Wo/LN2 step two blocks (so aoT's transpose is hidden).
        pending = attn_s(0)
        load_split(nc.sync, w1_t, w1)
        for b in range(1, NB):
            nxt = attn_s(b)
            attn_pv(b - 1, *pending)
            pending = nxt
            if b >= 2:
                phase_d(b - 2)
            if b == 2:
                load_split(nc.sync, w2_t, w2)
            if b == 5:
                ln2_batch(var3a, [0, 1, 2])
        attn_pv(NB - 1, *pending)
        phase_d(NB - 2)
        phase_d(NB - 1)
        ln2_batch(var3b, [3, 4, 5])

        # ---- FFN
        def ffn_in_group(g0):
            for fcx in range(FC):
                ps = psA.tile([P, 512], F32, tag="ps")
                for ci in range(DC):
                    nc.tensor.matmul(ps[:, :GR],
                                     w1_t[:, ci, fcx * P:(fcx + 1) * P],
                                     h2T[:, ci, g0:g0 + GR],
                                     start=(ci == 0), stop=(ci == DC - 1))
                gi = nc.scalar.activation(gt[:, fcx, g0:g0 + GR],
                                          ps[:, :GR], ACT.Gelu,
                                          bias=b1s_t[:, fcx:fcx + 1],
                                          scale=1.0)
                add_dep_helper(gi.ins, last_ht_mm[0].ins, sync=True,
                               reason="gt reuses HT slot")
                add_dep_helper(gi.ins, last_hnt_mm[0].ins, sync=True,
                               reason="gt overlays the mask tiles too")

        def ffn_out_block(b):
            ps = psA.tile([P, 512], F32, tag="ps")
            nc.tensor.matmul(ps, ones1, b2row_t, start=True, stop=False)
            for fcx in range(FC):
                nc.tensor.matmul(ps, gt[:, fcx, b * P:(b + 1) * P],
                                 w2_t[:, fcx, :],
                                 start=False, stop=(fcx == FC - 1))
            fo = temps.tile([P, D], F32, tag="fo")
            nc.vector.scalar_tensor_tensor(fo, ps, 0.0, x2t[:, b, :],
                                           op0=OP.bypass, op1=OP.add)
            nc.sync.dma_start(out=out[b * P:(b + 1) * P, :], in_=fo)

        ffn_in_group(0)
        for b in (0, 1, 2):
            ffn_out_block(b)
        ffn_in_group(GR)
        for b in (3, 4, 5):
            ffn_out_block(b)

    nc.compile()
    return nc


_prog = {}


def _get_program(H):
    if H not in _prog:
        _prog[H] = build_program(H)
    return _prog[H]


def _np_fallback(x, grid, Wq, bq, Wk, bk, Wv, bv, Wo, bo,
                 ln1_g, ln1_b, ln2_g, ln2_b, W1, b1, W2, b2):
    """Exact fp64 host path (only used if an input violates assumptions)."""
    from scipy.special import erf
    x = np.asarray(x, np.float64)
    g = np.asarray(grid).astype(np.float64)

    def ln(v, gm, bt, eps=1e-5):
        mu = v.mean(-1, keepdims=True)
        var = v.var(-1, keepdims=True)
        return (v - mu) / np.sqrt(var + eps) * gm + bt

    h = ln(x, ln1_g, ln1_b)
    q = h @ Wq + bq
    k = h @ Wk + bk
    v = h @ Wv + bv
    hn = h / np.maximum(np.linalg.norm(h, axis=-1, keepdims=True), 1e-8)
    scale = 1.0 / np.sqrt(D)
    n = x.shape[0]
    outp = np.empty_like(x)
    for s in range(0, n, 512):
        e = min(s + 512, n)
        dx = np.abs(g[s:e, None, 0] - g[None, :, 0])
        dy = np.abs(g[s:e, None, 1] - g[None, :, 1])
        mask = (dx <= RADIUS) & (dy <= RADIUS)
        mask[np.arange(e - s), np.arange(s, e)] = False
        logits = (q[s:e] @ k.T) * scale + hn[s:e] @ hn.T
        logits = np.where(mask, logits, -1e30)
        m = logits.max(-1, keepdims=True)
        p = np.exp(logits - m)
        att = p / p.sum(-1, keepdims=True)
        o = att @ v
        outp[s:e] = np.where(mask.any(1, keepdims=True), o, v[s:e])
    x = x + outp @ Wo + bo
    h2 = ln(x, ln2_g, ln2_b)
    a = h2 @ W1 + b1
    gelu = 0.5 * a * (1.0 + erf(a / np.sqrt(2.0)))
    return (x + gelu @ W2 + b2).astype(np.float32)


def kernel(x, grid, Wq, bq, Wk, bk, Wv, bv, Wo, bo,
           ln1_g, ln1_b, ln2_g, ln2_b, W1, b1, W2, b2):
    global LAST_EXEC_NS
    x = np.ascontiguousarray(np.asarray(x, np.float32))
    g = np.asarray(grid).astype(np.int64)

    affine1 = not (np.all(np.asarray(ln1_g) == 1.0)
                   and np.all(np.asarray(ln1_b) == 0.0))
    affine2 = not (np.all(np.asarray(ln2_g) == 1.0)
                   and np.all(np.asarray(ln2_b) == 0.0))

    # every node needs >=1 neighbour (else softmax sum underflows to 0)
    cells = np.zeros((96, 96), np.int64)
    np.add.at(cells, (g[:, 0], g[:, 1]), 1)
    import scipy.ndimage as ndi
    box = ndi.uniform_filter(cells.astype(np.float64), size=5, mode="constant")
    nbr = np.rint(box * 25.0).astype(np.int64)[g[:, 0], g[:, 1]] - 1
    isolated = np.any(nbr < 1)

    # ---- host-side sharding: sort rows by (gx, gy)
    perm = np.lexsort((g[:, 1], g[:, 0]))
    inv_perm = np.argsort(perm)
    gs = g[perm]
    xs = x[perm]

    # per-block halo requirement in sorted order
    gx = gs[:, 0]
    need = 0
    for b in range(N // P):
        lo = np.searchsorted(gx, gx[b * P:(b + 1) * P].min() - RADIUS, "left")
        hi = np.searchsorted(gx, gx[b * P:(b + 1) * P].max() + RADIUS, "right")
        need = max(need, b * P - lo, hi - (b * P + P))
    if need > 256 or isolated or affine1 or affine2:
        return _np_fallback(x, grid, Wq, bq, Wk, bk, Wv, bv, Wo, bo,
                            ln1_g, ln1_b, ln2_g, ln2_b, W1, b1, W2, b2)
    H = 192 if need <= 192 else 256
    KR = ROWS + 2 * H

    lam = 1.0 / np.sqrt(D)
    b16 = lambda a: np.ascontiguousarray(np.asarray(a)).astype(ml_dtypes.bfloat16)
    f32 = lambda a: np.ascontiguousarray(np.asarray(a, np.float32))

    # weights in [p, chunk, out] layout (p = contraction index % 128)
    chunked = lambda w_, nch: np.ascontiguousarray(
        b16(w_).reshape(nch, P, -1).transpose(1, 0, 2))
    wq_h = chunked(np.asarray(Wq, np.float64) * lam, DC)
    wk_h, wv_h, wo_h = (chunked(w_, DC) for w_ in (Wk, Wv, Wo))
    w1_h = chunked(W1, DC)
    w2_h = chunked(W2, FC)
    bqs_h = np.ascontiguousarray((np.asarray(bq, np.float64) * lam)
                                 .astype(np.float32).reshape(DC, P).T)
    bks_h = np.ascontiguousarray(f32(bk).reshape(DC, P).T)
    b1s_h = np.ascontiguousarray(f32(b1).reshape(FC, P).T)

    shared = dict(wq=wq_h, wk=wk_h, wv=wv_h, wo=wo_h, w1=w1_h, w2=w2_h,
                  iota=np.arange(P, dtype=np.float32).reshape(P, 1),
                  bqs=bqs_h, bks=bks_h, b1s=b1s_h,
                  bvrow=b16(bv).reshape(1, D),
                  borow=b16(bo).reshape(1, D),
                  b2row=b16(b2).reshape(1, D))

    xs_T16 = np.ascontiguousarray(xs.T).astype(ml_dtypes.bfloat16)  # [D, N]
    gs_f = gs.astype(np.float32)
    in_maps = []
    for c in range(N_CORES):
        glo = c * ROWS - H
        s0, s1 = max(0, glo), min(N, glo + KR)
        xt_c = np.zeros((D, KR), ml_dtypes.bfloat16)
        xt_c[:, s0 - glo:s1 - glo] = xs_T16[:, s0:s1]
        xtb_c = np.ascontiguousarray(
            xt_c.reshape(DC, P, KR).transpose(1, 0, 2))
        gk_c = np.full((KR, 2), 10000.0, np.float32)
        gk_c[s0 - glo:s1 - glo] = gs_f[s0:s1]
        gkb = gk_c.astype(ml_dtypes.bfloat16)
        in_maps.append(dict(shared,
                            xtb=xtb_c,
                            xq32=np.ascontiguousarray(
                                xs[c * ROWS:(c + 1) * ROWS]),
                            gxkb=np.ascontiguousarray(
                                np.broadcast_to(gkb[:, 0], (P, KR))),
                            gykb=np.ascontiguousarray(
                                np.broadcast_to(gkb[:, 1], (P, KR)))))

    nc = _get_program(H)
    tmpdir = os.environ.get("KERNEL_TRACE_DIR") or None
    res = run_bass_kernel_spmd(nc, in_maps, list(range(N_CORES)),
                               tmpdir=tmpdir)
    LAST_EXEC_NS = res.exec_time_ns
    out_sorted = np.concatenate([res.results[c]["out"]
                                 for c in range(N_CORES)], axis=0)
    return np.ascontiguousarray(out_sorted[inv_perm]).astype(np.float32)


# revision 24
# speedup vs baseline: 1.8799x; 1.8799x over previous
"""LocalBandSimilarityBlock — Trainium2 Bass kernel, 8-way sequence-parallel.

Strategy: sort nodes by grid-x (host-side sharding permutation). After the
sort, every node's radius-2 neighbourhood lies within a +-H-row band in
sorted order (verified per-input on host; H is chosen per input as 192 or
256). Each of the 8 cores owns 768 sorted query rows plus an H-row halo on
each side, so cores run fully independently, no collectives. Per 128-query
block, attention is computed against a (2H+128)-row key window instead of
all 6144 keys.

v2 layout: x arrives HOST-TRANSPOSED (xT, feature-major) so the LN1
normalisation happens directly in the transposed layout used by every
matmul — no on-device DMA transposes for h/hn. Row statistics (mean /
mean-square) are computed with ones-matmuls on the tensor engine, which
yields them already broadcast across partitions; 1/sqrt is computed as
exp(-0.5*ln(var+eps)) so LN1, softmax and LN2 share a single activation
table set (ln+exp) and only gelu needs a second table load. Softmax runs
without a max-subtraction: the additive band mask (+PEN per matching grid
axis via one-hot matmuls, -PEN on the self diagonal) is recentred by an
exp bias of -2*PEN, making valid logits O(1) and masked logits <= -PEN.
The whole schedule is software-pipelined: projections begin as soon as the
first 512 transposed-LN columns are ready, attention blocks interleave
with the Wo/LN2 phase, and the FFN starts while the attention tail is
still draining.
"""
import os
import sys

import numpy as np

sys.path.insert(0, "/opt/trn_rl_repo")

import ml_dtypes  # noqa: E402
from contextlib import ExitStack  # noqa: E402

import concourse.bacc as bacc  # noqa: E402
import concourse.tile as tile  # noqa: E402

# Force Ln and Exp onto their shared activation-table set: the table chooser
# is greedy-first-match over act_info.json order, which puts Ln in
# 'natural_log' and Exp in 'exp_and_others' and then thrashes a ~1.3us
# ACT_TABLE_LOAD on every Ln<->Exp switch.  Emptying the single-function
# sets (list order, and hence act_func_set_id numbering, is preserved)
# leaves 'natural_log_exp_and_others' as the only set serving either.
_orig_gat = bacc.get_activation_tables


def _patched_gat(arch):
    t = dict(_orig_gat(arch))
    for name in ("exp_and_others", "natural_log", "exp_and_friends"):
        if name in t:
            t[name] = type(t[name])()
    return t


bacc.get_activation_tables = _patched_gat
from concourse import mybir  # noqa: E402
from concourse.masks import make_identity  # noqa: E402
from concourse.tile_rust import add_dep_helper  # noqa: E402
from concourse.bass_utils import run_bass_kernel_spmd  # noqa: E402

P = 128
D = 512
DC = D // P            # 4 d-chunks
DFF = 2048
FC = DFF // P          # 16 ffn chunks
N = 6144
N_CORES = 8
ROWS = N // N_CORES    # 768 query rows per core
NB = ROWS // P         # 6 query blocks per core
GR = 384               # ffn row group (fits one PSUM bank)
PEN = 120.0
LN_EPS = 1e-5
RADIUS = 2
F32 = mybir.dt.float32
BF16 = mybir.dt.bfloat16
AX = mybir.AxisListType
OP = mybir.AluOpType
ACT = mybir.ActivationFunctionType
RNRB = float(-0.5 * np.log(D))   # exp bias turning rstd into 1/||h||

LAST_EXEC_NS = None


def build_program(H: int):
    KR = ROWS + 2 * H      # key rows per core
    NT = KR // P           # key chunks
    W = 2 * H + P          # key window per query block
    WSPANS = [(0, 512)] if W <= 512 else [(0, 512), (512, W - 512)]
    SL = [(s, min(512, KR - s)) for s in range(0, KR, 512)]  # stat col slices

    nc = bacc.Bacc("TRN2", target_bir_lowering=False, debug=False,
                   num_devices=N_CORES)
    dt_in = lambda name, shape, dt: nc.dram_tensor(name, shape, dt,
                                                   kind="ExternalInput").ap()
    xtb = dt_in("xtb", [P, DC, KR], BF16)    # host-transposed x (bf16)
    xq32 = dt_in("xq32", [ROWS, D], F32)     # own rows, row-major (residual)
    gxkb = dt_in("gxkb", [P, KR], BF16)
    gykb = dt_in("gykb", [P, KR], BF16)
    iota = dt_in("iota", [P, 1], F32)
    wq = dt_in("wq", [P, DC, D], BF16)       # pre-scaled by 1/sqrt(D) on host
    wk = dt_in("wk", [P, DC, D], BF16)
    wv = dt_in("wv", [P, DC, D], BF16)
    wo = dt_in("wo", [P, DC, D], BF16)
    w1 = dt_in("w1", [P, DC, DFF], BF16)
    w2 = dt_in("w2", [P, FC, D], BF16)
    bqs = dt_in("bqs", [P, DC], F32)         # (bq / sqrt(D)) chunked [p, dc]
    bks = dt_in("bks", [P, DC], F32)
    b1s = dt_in("b1s", [P, FC], F32)
    bvrow = dt_in("bvrow", [1, D], BF16)
    borow = dt_in("borow", [1, D], BF16)
    b2row = dt_in("b2row", [1, D], BF16)
    out = nc.dram_tensor("out", [ROWS, D], F32, kind="ExternalOutput").ap()

    with tile.TileContext(nc) as tc, ExitStack() as ctx:
        const = ctx.enter_context(tc.tile_pool(name="const", bufs=1))
        big = ctx.enter_context(tc.tile_pool(name="big", bufs=1))
        temps = ctx.enter_context(tc.tile_pool(name="temps", bufs=2))
        varp = ctx.enter_context(tc.tile_pool(name="varp", bufs=2))
        small = ctx.enter_context(tc.tile_pool(name="small", bufs=12))
        psA = ctx.enter_context(tc.tile_pool(name="psA", bufs=4, space="PSUM"))
        psS = ctx.enter_context(tc.tile_pool(name="psS", bufs=2, space="PSUM"))

        # float biases used by scalar.activation need registered const APs
        for val in (0.0, LN_EPS, -2.0 * PEN):
            ct = const.tile([P, 1], F32, tag=f"const_{val}")
            nc.vector.memset(ct, val)
            nc.const_aps.aps[(F32, val)] = ct

        # ---- transposed input first (everything is downstream of it);
        # 4 chunks, split columns to keep DMAs <= 256KB, first halves first
        # so slice-0 statistics can start after 4 transfers.
        xtb_t = big.tile([P, DC, KR], BF16)
        hkr = KR // 2
        for h0, h1 in [(0, hkr), (hkr, KR)]:
            for c in range(DC):
                nc.sync.dma_start(out=xtb_t[:, c, h0:h1], in_=xtb[:, c, h0:h1])

        # ---- small constants
        bqs_t = const.tile([P, DC], F32)
        bks_t = const.tile([P, DC], F32)
        b1s_t = const.tile([P, FC], F32)
        bvrow_t = const.tile([1, D], BF16)
        borow_t = const.tile([1, D], BF16)
        b2row_t = const.tile([1, D], BF16)
        iota_t = const.tile([P, 1], F32)
        for t_, a_ in [(iota_t, iota), (bks_t, bks), (bqs_t, bqs),
                       (bvrow_t, bvrow), (borow_t, borow), (b2row_t, b2row),
                       (b1s_t, b1s)]:
            nc.sync.dma_start(out=t_, in_=a_)

        gx_st = temps.tile([P, KR], BF16, tag="gstage", bufs=2)
        gy_st = temps.tile([P, KR], BF16, tag="gstage", bufs=2)
        nc.sync.dma_start(out=gx_st, in_=gxkb)
        nc.sync.dma_start(out=gy_st, in_=gykb)

        def load_split(eng, dst, srcap, max_bytes=262144):
            """DMA in <=256KB pieces (one HW sub-DMA each)."""
            nbytes = int(np.prod(dst.shape)) * mybir.dt.size(dst.dtype)
            if nbytes <= max_bytes or len(dst.shape) < 2:
                eng.dma_start(out=dst, in_=srcap)
                return
            nsplit = -(-nbytes // max_bytes)
            cnt = dst.shape[1]
            step = max(1, -(-cnt // nsplit))
            for s0 in range(0, cnt, step):
                s1 = min(cnt, s0 + step)
                if len(dst.shape) == 2:
                    eng.dma_start(out=dst[:, s0:s1], in_=srcap[:, s0:s1])
                else:
                    eng.dma_start(out=dst[:, s0:s1, :], in_=srcap[:, s0:s1, :])

        wq_t = const.tile([P, DC, D], BF16)
        wk_t = const.tile([P, DC, D], BF16)
        wv_t = const.tile([P, DC, D], BF16)
        wo_t = const.tile([P, DC, D], BF16)
        w1_t = const.tile([P, DC, DFF], BF16)
        w2_t = const.tile([P, FC, D], BF16)
        for t_, a_ in [(wk_t, wk), (wv_t, wv), (wq_t, wq)]:
            load_split(nc.scalar, t_, a_)

        # identity / shifted self-penalty / ones
        ident = const.tile([P, P], BF16)
        make_identity(nc, ident)
        shiftpen = const.tile([P, W], BF16)
        nc.gpsimd.memset(shiftpen, 0.0)
        nc.gpsimd.affine_select(out=shiftpen, in_=shiftpen,
                                compare_op=OP.not_equal, fill=-PEN,
                                base=H, channel_multiplier=1,
                                pattern=[[-1, W]])
        ones1 = const.tile([1, P], BF16)
        nc.vector.memset(ones1, 1.0)
        onesD = const.tile([P, P], BF16)
        nc.vector.memset(onesD, 1.0 / D)

        # ---- persistent activation tiles.  HT and the 4 band-mask tiles
        # pack into one [P, 8, KR] slab that gt later reuses (all are dead
        # once the last attention block's S accumulation has run).
        slabA = big.tile([P, 8, KR], BF16, tag="slotA")
        HT = slabA[:, 0:DC, :]
        ExPEN, EyPEN = slabA[:, 4, :], slabA[:, 5, :]
        Fx, Fy = slabA[:, 6, :], slabA[:, 7, :]
        gt = big.tile([P, FC, ROWS], BF16, tag="slotA")   # reuses slabA
        xsq = big.tile([P, DC, KR], BF16, tag="slotB")
        x2t = big.tile([P, NB, D], F32, tag="slotB")      # reuses xsq slot
        # hn^T = h^T / ||h||, and ||h|| == sqrt(D*(var+eps))*rstd is the
        # SAME normalisation as h up to the constant 1/sqrt(D) — so the
        # sim term hn_i.hn_j contracts hnq (query side, = HT/D) against
        # HT itself on the key side.  No separate hn tensor needed.
        hnq = big.tile([P, DC, ROWS], BF16)

        kt = big.tile([P, DC, KR], BF16)
        qt = big.tile([P, DC, ROWS], BF16)
        vt = big.tile([P, NT, D], BF16)
        aoT = big.tile([P, DC, ROWS], BF16)
        h2T = big.tile([P, DC, ROWS], BF16)
        mu_bc = big.tile([P, KR], BF16)
        rstd_bc = big.tile([P, KR], BF16)
        var3a = small.tile([P, 3], F32, tag="var3a")
        var3b = small.tile([P, 3], F32, tag="var3b")
        mv6 = [small.tile([P, 2], F32, tag=f"mv6_{b}", name=f"mv6_{b}")
               for b in range(NB)]

        # x^2 for the mean-square statistic — first thing on the vector
        # queue (the whole kernel is downstream of the statistics)
        for c in range(DC):
            nc.vector.tensor_tensor(xsq[:, c, :], xtb_t[:, c, :],
                                    xtb_t[:, c, :], op=OP.mult)

        last_xsq_mm = [None]
        last_ht_mm = [None]
        last_hnt_mm = [None]

        # ---- LN1 in transposed land, per 512-column slice: statistics part
        # (ones-matmuls give mean / mean-square already broadcast across
        # partitions; 1/sqrt via ln+exp so softmax shares the table set)
        def ln_stats(s0, sn):
            mu_ps = psA.tile([P, 512], F32, tag="ps")
            for c in range(DC):
                nc.tensor.matmul(mu_ps[:, :sn], onesD,
                                 xtb_t[:, c, s0:s0 + sn],
                                 start=(c == 0), stop=(c == DC - 1))
            mq_ps = psA.tile([P, 512], F32, tag="ps")
            for c in range(DC):
                mm = nc.tensor.matmul(mq_ps[:, :sn], onesD,
                                      xsq[:, c, s0:s0 + sn],
                                      start=(c == 0), stop=(c == DC - 1))
                last_xsq_mm[0] = mm
            vs = varp.tile([P, 512], F32, tag="vs")
            nc.scalar.activation(vs[:, :sn], mu_ps[:, :sn], ACT.Square)
            nc.vector.tensor_tensor(vs[:, :sn], mq_ps[:, :sn], vs[:, :sn],
                                    op=OP.subtract)
            nc.vector.tensor_scalar_add(mu_bc[:, s0:s0 + sn], mu_ps[:, :sn],
                                        0.0)
            lnv = varp.tile([P, 512], F32, tag="lnv")
            nc.scalar.activation(lnv[:, :sn], vs[:, :sn], ACT.Ln, bias=LN_EPS)
            nc.scalar.activation(rstd_bc[:, s0:s0 + sn], lnv[:, :sn],
                                 ACT.Exp, scale=-0.5)

        def ln_ht(s0, sn):
            for c in range(DC):
                tcc = temps.tile([P, 512], BF16, tag="tcc")
                nc.vector.tensor_tensor(tcc[:, :sn], xtb_t[:, c, s0:s0 + sn],
                                        mu_bc[:, s0:s0 + sn], op=OP.subtract)
                nc.vector.tensor_tensor(HT[:, c, s0:s0 + sn], tcc[:, :sn],
                                        rstd_bc[:, s0:s0 + sn], op=OP.mult)

        def build_masks():
            # band-mask one-hot encodings (axis match -> +PEN via matmul)
            for src_bc, Et, Ft in [(gx_st, ExPEN, Fx), (gy_st, EyPEN, Fy)]:
                nc.vector.tensor_scalar(Et, src_bc, iota_t, PEN,
                                        op0=OP.is_equal, op1=OP.mult)
                u = temps.tile([P, KR], BF16, tag="uband")
                nc.vector.tensor_scalar(u, src_bc, iota_t, None,
                                        op0=OP.subtract)
                a1 = temps.tile([P, KR], BF16, tag="uband")
                nc.vector.tensor_scalar(a1, u, RADIUS + 0.5, None,
                                        op0=OP.is_le)
                nc.vector.tensor_scalar(u, u, -(RADIUS + 0.5), None,
                                        op0=OP.is_ge)
                nc.vector.tensor_tensor(Ft, a1, u, op=OP.mult)

        def kq_rowblock(kind, r0, rn_):
            """q^T/k^T row-block: 4 dout-chunks x (4 accum + bias) matmuls."""
            src_t, dst, bias, off = ((wk_t, kt, bks_t, 0) if kind == "k"
                                     else (wq_t, qt, bqs_t, H))
            for dcx in range(DC):
                ps = psA.tile([P, 512], F32, tag="ps")
                for ci in range(DC):
                    mm = nc.tensor.matmul(ps[:, :rn_],
                                          src_t[:, ci, dcx * P:(dcx + 1) * P],
                                          HT[:, ci, off + r0:off + r0 + rn_],
                                          start=(ci == 0), stop=(ci == DC - 1))
                    last_ht_mm[0] = mm
                nc.scalar.activation(dst[:, dcx, r0:r0 + rn_],
                                     ps[:, :rn_], ACT.Identity,
                                     bias=bias[:, dcx:dcx + 1], scale=1.0)

        def v_chunk(t):
            """v rows t*128..t*128+128 (row-major), bias folded as K=1 mm."""
            ps = psA.tile([P, 512], F32, tag="ps")
            for ci in range(DC):
                mm = nc.tensor.matmul(ps, HT[:, ci, t * P:(t + 1) * P],
                                      wv_t[:, ci, :], start=(ci == 0),
                                      stop=False)
                last_ht_mm[0] = mm
            nc.tensor.matmul(ps, ones1, bvrow_t, start=False, stop=True)
            nc.vector.tensor_scalar_add(vt[:, t, :], ps, 0.0)

        # pipelined emission: all statistics first (tensor runs them while
        # the vector/scalar chain catches up), then per-slice normalised
        # columns followed by the projections that only need that slice.
        for s0, sn in SL:
            ln_stats(s0, sn)
        ln_ht(*SL[0])
        kq_rowblock("k", 0, 512)
        for t in range(0, 4):
            v_chunk(t)
        ln_ht(*SL[1])
        for c in range(DC):   # query-side hn = HT/D (see hnq comment)
            nc.vector.tensor_scalar_mul(hnq[:, c, :], HT[:, c, H:H + ROWS],
                                        1.0 / D)
        build_masks()
        kq_rowblock("q", 0, 512)
        kq_rowblock("k", 512, 512)
        kq_rowblock("q", 512, 256)
        for t in range(4, 8):
            v_chunk(t)
        if len(SL) > 2:
            ln_ht(*SL[2])
        if KR > 1024:
            kq_rowblock("k", 1024, KR - 1024)
        for t in range(8, NT):
            v_chunk(t)
        load_split(nc.scalar, wo_t, wo)

        # ---- attention block: S accum -> exp(bias=-2PEN) -> P^T -> PV
        def attn_s(b):
            wb = P * b
            S = psS.tile([P, W], F32, tag="S")
            for c0, cn in WSPANS:
                nc.tensor.matmul(S[:, c0:c0 + cn], ident,
                                 shiftpen[:, c0:c0 + cn], start=True,
                                 stop=False)
                for Et, Ft in [(ExPEN, Fx), (EyPEN, Fy)]:
                    nc.tensor.matmul(S[:, c0:c0 + cn],
                                     Et[:, H + wb:H + wb + P],
                                     Ft[:, wb + c0:wb + c0 + cn],
                                     start=False, stop=False)
                for ac in range(2 * DC):
                    if ac < DC:
                        lhsT = qt[:, ac, wb:wb + P]
                        rhs = kt[:, ac, wb + c0:wb + c0 + cn]
                    else:
                        lhsT = hnq[:, ac - DC, wb:wb + P]
                        rhs = HT[:, ac - DC, wb + c0:wb + c0 + cn]
                    mm = nc.tensor.matmul(
                        S[:, c0:c0 + cn], lhsT, rhs, start=False,
                        stop=(ac == 2 * DC - 1))
                    if ac >= DC:
                        last_hnt_mm[0] = mm
            pb = temps.tile([P, W], BF16, tag="pb")
            srow = small.tile([P, 1], F32, tag="srow")
            nc.scalar.activation(pb, S, ACT.Exp, bias=-2.0 * PEN, scale=1.0,
                                 accum_out=srow)
            rs = small.tile([P, 1], F32, tag="rs")
            nc.vector.reciprocal(rs, srow)
            PT = temps.tile([P, W // P, P], BF16, tag="PT")
            nc.sync.dma_start_transpose(PT[:, :, :], pb)
            return PT, rs

        def attn_pv(b, PT, rs):
            po = psA.tile([P, 512], F32, tag="ps")
            for j in range(W // P):
                nc.tensor.matmul(po, PT[:, j, :], vt[:, b + j, :],
                                 start=(j == 0), stop=(j == W // P - 1))
            aob = temps.tile([P, D], BF16, tag="aob")
            nc.vector.tensor_scalar_mul(aob, po, rs)
            nc.scalar.dma_start_transpose(aoT[:, :, b * P:(b + 1) * P], aob)

        def phase_d(b):
            """x2 = x + attn@Wo + bo, then LN2 stats (no scalar table ops)."""
            xr = temps.tile([P, D], F32, tag="xr")
            nc.gpsimd.dma_start(out=xr, in_=xq32[b * P:(b + 1) * P, :])
            ps = psA.tile([P, 512], F32, tag="ps")
            nc.tensor.matmul(ps, ones1, borow_t, start=True, stop=False)
            for ci in range(DC):
                nc.tensor.matmul(ps, aoT[:, ci, b * P:(b + 1) * P],
                                 wo_t[:, ci, :],
                                 start=False, stop=(ci == DC - 1))
            x2 = x2t[:, b, :]
            x2i = nc.vector.scalar_tensor_tensor(x2, ps, 0.0, xr,
                                                 op0=OP.bypass, op1=OP.add)
            add_dep_helper(x2i.ins, last_xsq_mm[0].ins, sync=True,
                           reason="x2t reuses xsq slot")
            st = small.tile([P, 6], F32, tag="st")
            nc.vector.bn_stats(st, x2)
            nc.vector.bn_aggr(mv6[b], st)
            dst = var3a if b < 3 else var3b
            nc.vector.tensor_scalar_add(dst[:, b % 3:b % 3 + 1],
                                        mv6[b][:, 1:2], 0.0)

        def ln2_batch(var3, blocks):
            lnv3 = small.tile([P, 3], F32, tag="lnv3")
            nc.scalar.activation(lnv3, var3, ACT.Ln, bias=LN_EPS)
            rstd3 = small.tile([P, 3], F32, tag="rstd3")
            nc.scalar.activation(rstd3, lnv3, ACT.Exp, scale=-0.5)
            for i, b in enumerate(blocks):
                h2b = temps.tile([P, D], BF16, tag="h2b")
                nc.vector.tensor_scalar(h2b, x2t[:, b, :], mv6[b][:, 0:1],
                                        rstd3[:, i:i + 1],
                                        op0=OP.subtract, op1=OP.mult)
                eng = nc.sync if b % 2 == 0 else nc.scalar
                eng.dma_start_transpose(h2T[:, :, b * P:(b + 1) * P], h2b)

        # ---- attention + phase D interleaved: PV lags one block behind S,
        # the Wo/LN2 step two blocks (so aoT's transpose is hidden).
        pending = attn_s(0)
        load_split(nc.sync, w1_t, w1)
        for b in range(1, NB):
            nxt = attn_s(b)
            attn_pv(b - 1, *pending)
            pending = nxt
            if b >= 2:
                phase_d(b - 2)
            if b == 2:
                load_split(nc.sync, w2_t, w2)
            if b == 5:
                ln2_batch(var3a, [0, 1, 2])
        attn_pv(NB - 1, *pending)
        phase_d(NB - 2)
        phase_d(NB - 1)
        ln2_batch(var3b, [3, 4, 5])

        # ---- FFN
        def ffn_in_group(g0):
            for fcx in range(FC):
                ps = psA.tile([P, 512], F32, tag="ps")
                for ci in range(DC):
                    nc.tensor.matmul(ps[:, :GR],
                                     w1_t[:, ci, fcx * P:(fcx + 1) * P],
                                     h2T[:, ci, g0:g0 + GR],
                                     start=(ci == 0), stop=(ci == DC - 1))
                gi = nc.scalar.activation(gt[:, fcx, g0:g0 + GR],
                                          ps[:, :GR], ACT.Gelu,
                                          bias=b1s_t[:, fcx:fcx + 1],
                                          scale=1.0)
                add_dep_helper(gi.ins, last_ht_mm[0].ins, sync=True,
                               reason="gt reuses HT slot")
                add_dep_helper(gi.ins, last_hnt_mm[0].ins, sync=True,
                               reason="gt overlays the mask tiles too")

        def ffn_out_block(b):
            ps = psA.tile([P, 512], F32, tag="ps")
            nc.tensor.matmul(ps, ones1, b2row_t, start=True, stop=False)
            for fcx in range(FC):
                nc.tensor.matmul(ps, gt[:, fcx, b * P:(b + 1) * P],
                                 w2_t[:, fcx, :],
                                 start=False, stop=(fcx == FC - 1))
            fo = temps.tile([P, D], F32, tag="fo")
            nc.vector.scalar_tensor_tensor(fo, ps, 0.0, x2t[:, b, :],
                                           op0=OP.bypass, op1=OP.add)
            nc.sync.dma_start(out=out[b * P:(b + 1) * P, :], in_=fo)

        ffn_in_group(0)
        for b in (0, 1, 2):
            ffn_out_block(b)
        ffn_in_group(GR)
        for b in (3, 4, 5):
            ffn_out_block(b)

    nc.compile()
    return nc


_prog = {}


def _get_program(H):
    if H not in _prog:
        _prog[H] = build_program(H)
    return _prog[H]


def _np_fallback(x, grid, Wq, bq, Wk, bk, Wv, bv, Wo, bo,
                 ln1_g, ln1_b, ln2_g, ln2_b, W1, b1, W2, b2):
    """Exact fp64 host path (only used if an input violates assumptions)."""
    from scipy.special import erf
    x = np.asarray(x, np.float64)
    g = np.asarray(grid).astype(np.float64)

    def ln(v, gm, bt, eps=1e-5):
        mu = v.mean(-1, keepdims=True)
        var = v.var(-1, keepdims=True)
        return (v - mu) / np.sqrt(var + eps) * gm + bt

    h = ln(x, ln1_g, ln1_b)
    q = h @ Wq + bq
    k = h @ Wk + bk
    v = h @ Wv + bv
    hn = h / np.maximum(np.linalg.norm(h, axis=-1, keepdims=True), 1e-8)
    scale = 1.0 / np.sqrt(D)
    n = x.shape[0]
    outp = np.empty_like(x)
    for s in range(0, n, 512):
        e = min(s + 512, n)
        dx = np.abs(g[s:e, None, 0] - g[None, :, 0])
        dy = np.abs(g[s:e, None, 1] - g[None, :, 1])
        mask = (dx <= RADIUS) & (dy <= RADIUS)
        mask[np.arange(e - s), np.arange(s, e)] = False
        logits = (q[s:e] @ k.T) * scale + hn[s:e] @ hn.T
        logits = np.where(mask, logits, -1e30)
        m = logits.max(-1, keepdims=True)
        p = np.exp(logits - m)
        att = p / p.sum(-1, keepdims=True)
        o = att @ v
        outp[s:e] = np.where(mask.any(1, keepdims=True), o, v[s:e])
    x = x + outp @ Wo + bo
    h2 = ln(x, ln2_g, ln2_b)
    a = h2 @ W1 + b1
    gelu = 0.5 * a * (1.0 + erf(a / np.sqrt(2.0)))
    return (x + gelu @ W2 + b2).astype(np.float32)


def kernel(x, grid, Wq, bq, Wk, bk, Wv, bv, Wo, bo,
           ln1_g, ln1_b, ln2_g, ln2_b, W1, b1, W2, b2):
    global LAST_EXEC_NS
    x = np.ascontiguousarray(np.asarray(x, np.float32))
    g = np.asarray(grid).astype(np.int64)

    affine1 = not (np.all(np.asarray(ln1_g) == 1.0)
                   and np.all(np.asarray(ln1_b) == 0.0))
    affine2 = not (np.all(np.asarray(ln2_g) == 1.0)
                   and np.all(np.asarray(ln2_b) == 0.0))

    # every node needs >=1 neighbour (else softmax sum underflows to 0)
    cells = np.zeros((96, 96), np.int64)
    np.add.at(cells, (g[:, 0], g[:, 1]), 1)
    import scipy.ndimage as ndi
    box = ndi.uniform_filter(cells.astype(np.float64), size=5, mode="constant")
    nbr = np.rint(box * 25.0).astype(np.int64)[g[:, 0], g[:, 1]] - 1
    isolated = np.any(nbr < 1)

    # ---- host-side sharding: sort rows by (gx, gy)
    perm = np.lexsort((g[:, 1], g[:, 0]))
    inv_perm = np.argsort(perm)
    gs = g[perm]
    xs = x[perm]

    # per-block halo requirement in sorted order
    gx = gs[:, 0]
    need = 0
    for b in range(N // P):
        lo = np.searchsorted(gx, gx[b * P:(b + 1) * P].min() - RADIUS, "left")
        hi = np.searchsorted(gx, gx[b * P:(b + 1) * P].max() + RADIUS, "right")
        need = max(need, b * P - lo, hi - (b * P + P))
    if need > 256 or isolated or affine1 or affine2:
        return _np_fallback(x, grid, Wq, bq, Wk, bk, Wv, bv, Wo, bo,
                            ln1_g, ln1_b, ln2_g, ln2_b, W1, b1, W2, b2)
    H = 192 if need <= 192 else 256
    KR = ROWS + 2 * H

    lam = 1.0 / np.sqrt(D)
    b16 = lambda a: np.ascontiguousarray(np.asarray(a)).astype(ml_dtypes.bfloat16)
    f32 = lambda a: np.ascontiguousarray(np.asarray(a, np.float32))

    # weights in [p, chunk, out] layout (p = contraction index % 128)
    chunked = lambda w_, nch: np.ascontiguousarray(
        b16(w_).reshape(nch, P, -1).transpose(1, 0, 2))
    wq_h = chunked(np.asarray(Wq, np.float64) * lam, DC)
    wk_h, wv_h, wo_h = (chunked(w_, DC) for w_ in (Wk, Wv, Wo))
    w1_h = chunked(W1, DC)
    w2_h = chunked(W2, FC)
    bqs_h = np.ascontiguousarray((np.asarray(bq, np.float64) * lam)
                                 .astype(np.float32).reshape(DC, P).T)
    bks_h = np.ascontiguousarray(f32(bk).reshape(DC, P).T)
    b1s_h = np.ascontiguousarray(f32(b1).reshape(FC, P).T)

    shared = dict(wq=wq_h, wk=wk_h, wv=wv_h, wo=wo_h, w1=w1_h, w2=w2_h,
                  iota=np.arange(P, dtype=np.float32).reshape(P, 1),
                  bqs=bqs_h, bks=bks_h, b1s=b1s_h,
                  bvrow=b16(bv).reshape(1, D),
                  borow=b16(bo).reshape(1, D),
                  b2row=b16(b2).reshape(1, D))

    xs_T16 = np.ascontiguousarray(xs.T).astype(ml_dtypes.bfloat16)  # [D, N]
    gs_f = gs.astype(np.float32)
    in_maps = []
    for c in range(N_CORES):
        glo = c * ROWS - H
        s0, s1 = max(0, glo), min(N, glo + KR)
        xt_c = np.zeros((D, KR), ml_dtypes.bfloat16)
        xt_c[:, s0 - glo:s1 - glo] = xs_T16[:, s0:s1]
        xtb_c = np.ascontiguousarray(
            xt_c.reshape(DC, P, KR).transpose(1, 0, 2))
        gk_c = np.full((KR, 2), 10000.0, np.float32)
        gk_c[s0 - glo:s1 - glo] = gs_f[s0:s1]
        gkb = gk_c.astype(ml_dtypes.bfloat16)
        in_maps.append(dict(shared,
                            xtb=xtb_c,
                            xq32=np.ascontiguousarray(
                                xs[c * ROWS:(c + 1) * ROWS]),
                            gxkb=np.ascontiguousarray(
                                np.broadcast_to(gkb[:, 0], (P, KR))),
                            gykb=np.ascontiguousarray(
                                np.broadcast_to(gkb[:, 1], (P, KR)))))

    nc = _get_program(H)
    tmpdir = os.environ.get("KERNEL_TRACE_DIR") or None
    res = run_bass_kernel_spmd(nc, in_maps, list(range(N_CORES)),
                               tmpdir=tmpdir)
    LAST_EXEC_NS = res.exec_time_ns
    out_sorted = np.concatenate([res.results[c]["out"]
                                 for c in range(N_CORES)], axis=0)
    return np.ascontiguousarray(out_sorted[inv_perm]).astype(np.float32)
